# revision 43
# baseline (speedup 1.0000x reference)
"""AnomalyMapGenerator Trainium2 kernel.

Reference computation: nearest-neighbor upsample of patch_scores
[B=32,1,28,28] -> [B,1,512,512], then a dense 33x33 blur conv (padding 16),
then mean over the (singleton) channel dim -> [B,512,512].

Both stages are linear and separable along H and W, so the whole map
collapses to  out[b] = A @ s[b] @ B^T  with A, B of shape [512, 28]:

    up = U s U^T            (U [512,28] is the 0/1 nearest-upsample matrix)
    out = C_h up C_w^T      (C_* [512,512] Toeplitz matrices of the 1-D taps)
    =>  out = (C_h U) s (C_w U)^T = A s B^T

The 33x33 blur weight is factored into separable 1-D taps by SVD on the host
(it is an exact rank-1 Gaussian outer product; general rank-R kernels are
handled by summing rank-1 terms in PSUM). The heavy work - 32 images of
[512,28]@[28,28] and [512,28]@[28,512] matmuls plus the 128 MiB output
write - runs on 8 NeuronCores, batch-sharded 4 images per core.
"""

import numpy as np

# ---- problem geometry (hardcoded per spec) ---------------------------------
B_FULL = 32
SH = 28          # source patch side
H = 512          # output side
KS = 33          # blur kernel side
PAD = KS // 2
N_CORES = 8
PB = B_FULL // N_CORES   # images per core
M_CHUNKS = H // 128      # output row chunks per image
MAX_RG = 4               # max rank-1 blur terms processed per device pass

_cache = {}


def _factor_blur(blur_w):
    """Host-side weight packing: factor the 2-D blur kernel into rank-1
    separable terms and fold each with the nearest-upsample matrix.

    Returns (AT, BT, R): AT/BT are [R*28, 512] f32, where
    AT[r*28:(r+1)*28] = A_r^T and out = sum_r A_r s B_r^T.
    """
    w2d = np.asarray(blur_w, dtype=np.float64).reshape(KS, KS)
    uu, sv, vt = np.linalg.svd(w2d)
    R = max(1, int(np.sum(sv > sv[0] * 1e-6))) if sv[0] > 0 else 1

    idx = np.arange(H)
    U = np.zeros((H, SH))
    U[idx, (idx * SH) // H] = 1.0
    # C[y, Y] = k[Y - y + PAD] for |Y - y| <= PAD (cross-correlation, zero pad)
    D = idx[None, :] - idx[:, None] + PAD
    valid = (D >= 0) & (D <= KS - 1)
    Dc = np.clip(D, 0, KS - 1)

    ats, bts = [], []
    for r in range(R):
        A = np.where(valid, np.take(uu[:, r] * sv[r], Dc), 0.0) @ U   # [512, 28]
        Bm = np.where(valid, np.take(vt[r, :], Dc), 0.0) @ U          # [512, 28]
        ats.append(np.ascontiguousarray(A.T))
        bts.append(np.ascontiguousarray(Bm.T))
    AT = np.concatenate(ats, axis=0).astype(np.float32)  # [R*28, 512]
    BT = np.concatenate(bts, axis=0).astype(np.float32)  # [R*28, 512]
    return AT, BT, R


def _build_nc(R):
    """Per-core Bass graph: out[b] = sum_r A_r s_b B_r^T for PB images.

    mm1:  t_r^T [28,512] = lhsT(s_b [28i,28j]).T @ rhs(A_r^T [28i,512y])
    mm2:  out_c [128,512] += lhsT(t_r^T[:, c*128:+128]).T @ rhs(B_r^T [28j,512x])
    """
    import concourse.mybir as mybir
    from concourse import bacc
    from concourse.tile import TileContext

    f32 = mybir.dt.float32
    bf16 = mybir.dt.bfloat16
    # float32r: same 4-byte storage as f32, but the PE runs it at full rate
    # (1 cycle/row at N>=256) instead of fp32's 4 cycles/row
    f32r = mybir.dt.float32r
    nc = bacc.Bacc("TRN2", target_bir_lowering=False, debug=False,
                   num_devices=N_CORES)

    # packed input [64, R*512 (A^T) | 128 (s, per-pair 32-strided) | R*512
    # (B^T, replicated at partition groups 0 and 32)]: a pair of images runs
    # its two mm2s on disjoint PE row groups concurrently, which requires
    # fmap (B^T) and weights (t^T) to share a base partition. Two DMAs
    # (mm1 operands first) cut the to-first-matmul latency.
    FW = 2 * R * H + 128
    FW1 = R * H + 128
    inp_d = nc.declare_dram_parameter("inp", [64, FW], f32r, isOutput=False)
    # output is staged and streamed to HBM as bf16 (halves the dominant
    # HBM-write cost); the host upcasts to f32. Output quantization error
    # ~2e-3 fro-rel, well inside the accuracy gate.
    out_d = nc.declare_dram_parameter("out", [PB, H, H], bf16, isOutput=True)

    with TileContext(nc) as tc:
        with (
            tc.tile_pool(name="const", bufs=1) as cpool,
            tc.tile_pool(name="tt", bufs=2) as tpool,
            tc.tile_pool(name="pt", bufs=2, space="PSUM") as pt_pool,
            tc.tile_pool(name="po", bufs=6, space="PSUM") as po_pool,
            tc.tile_pool(name="ob", bufs=4) as opool,
        ):
            inp_t = cpool.tile([64, FW], f32r, tag="inp")
            # one DMA for all inputs: a second dma_start would serialize
            # ~0.7us of issue time on Sync ahead of the first output piece
            nc.sync.dma_start(out=inp_t[:], in_=inp_d[:, :])
            at_t = inp_t[:SH, 0:R * H]
            s_t = inp_t[:SH, R * H:FW1]  # [28, 128]: pair P at cols P*64
            bt_t = inp_t[:, FW1:]        # [64, R*512]: B^T at groups 0, 32

            for P in range(PB // 2):
                # staging for both images of the pair: (b2, c, x) layout
                ob_t = opool.tile([128, 2 * M_CHUNKS * H], bf16, tag="ob")
                tts = []
                for r in range(R):
                    # one mm1 covers the pair: lhsT [28, 64] -> both t^T at
                    # 32-aligned partition groups of one PSUM tile
                    pt_t = pt_pool.tile([64, H], f32, tag="pt")
                    nc.tensor.matmul(
                        out=pt_t[:],
                        lhsT=s_t[:, P * 64:(P + 1) * 64],
                        rhs=at_t[:, r * H:(r + 1) * H],
                        start=True, stop=True,
                    )
                    tt_t = tpool.tile([64, H], f32r, tag=f"tt{r}")
                    # cast per y-chunk column so each mm2 pair waits on one
                    for c in range(M_CHUNKS):
                        piece = (slice(None), slice(c * 128, (c + 1) * 128))
                        if (c + r) % 2 == 0:
                            nc.vector.tensor_copy(out=tt_t[piece],
                                                  in_=pt_t[piece])
                        else:
                            nc.scalar.copy(out=tt_t[piece], in_=pt_t[piece])
                    tts.append(tt_t)
                for c in range(M_CHUNKS):
                    pos = []
                    for h2 in range(2):  # image P*2 + h2
                        po_t = po_pool.tile([128, H], f32, tag="po",
                                            name=f"po_{P}_{c}_{h2}")
                        for r in range(R):
                            # row groups 0 / 32 -> the pair's two matmuls
                            # execute concurrently in the PE array
                            nc.tensor.matmul(
                                out=po_t[:],
                                lhsT=tts[r][h2 * 32:h2 * 32 + SH,
                                            c * 128:(c + 1) * 128],
                                rhs=bt_t[h2 * 32:h2 * 32 + SH,
                                         r * H:(r + 1) * H],
                                start=(r == 0), stop=(r == R - 1),
                                tile_position=(h2 * 32, 0),
                            )
                        pos.append(po_t)
                    # the pair's two copies run on both engines in parallel,
                    # then the (pair, chunk) leaves as one 256 KiB DMA. The
                    # very first piece gates the HBM stream start, so its two
                    # copies are split into engine-parallel halves.
                    for h2 in range(2):
                        dst = ob_t[:, (h2 * M_CHUNKS + c) * H:
                                   (h2 * M_CHUNKS + c + 1) * H]
                        if P == 0 and c == 0:
                            half = H // 2
                            if h2 == 0:
                                nc.vector.tensor_copy(out=dst[:, :half],
                                                      in_=pos[h2][:, :half])
                                nc.scalar.copy(out=dst[:, half:],
                                               in_=pos[h2][:, half:])
                            else:
                                nc.scalar.copy(out=dst[:, :half],
                                               in_=pos[h2][:, :half])
                                nc.vector.tensor_copy(out=dst[:, half:],
                                                      in_=pos[h2][:, half:])
                        elif h2 == 0:
                            nc.vector.tensor_copy(out=dst, in_=pos[h2][:])
                        else:
                            nc.scalar.copy(out=dst, in_=pos[h2][:])
                    nc.sync.dma_start(
                        out=out_d[2 * P:2 * P + 2, c * 128:(c + 1) * 128, :]
                            .rearrange("b p x -> p b x"),
                        in_=ob_t[:].rearrange("p (b c x) -> p b c x",
                                              b=2, x=H)[:, :, c, :],
                    )
    nc.compile()
    return nc


def _get_nc(R):
    key = ("nc", R)
    if key not in _cache:
        _cache[key] = _build_nc(R)
    return _cache[key]


def _pack_in_maps(ps, AT, BT):
    """Pack per-core inputs [64, R*512 | 128 | R*512] for one rank group.

    s columns for image b sit at (b//2)*64 + (b%2)*32 + j; B^T is replicated
    at partition groups 0 and 32 for the row-packed mm2 pairs.
    """
    R = AT.shape[0] // SH
    at_cols = np.concatenate([AT[r * SH:(r + 1) * SH] for r in range(R)], axis=1)
    bt_cols = np.concatenate([BT[r * SH:(r + 1) * SH] for r in range(R)], axis=1)
    RH = R * H
    in_maps = []
    for i in range(N_CORES):
        inp = np.zeros((64, 2 * RH + 128), np.float32)
        inp[:SH, :RH] = at_cols
        for b in range(PB):
            col = RH + (b // 2) * 64 + (b % 2) * 32
            inp[:SH, col:col + SH] = ps[i * PB + b]  # [i, j]
        inp[0:SH, RH + 128:] = bt_cols
        inp[32:32 + SH, RH + 128:] = bt_cols
        in_maps.append({"inp": np.ascontiguousarray(inp)})
    return in_maps, R


def _make_in_maps(patch_scores, blur_w):
    ps = np.asarray(patch_scores, dtype=np.float32).reshape(B_FULL, SH, SH)
    AT, BT, R = _factor_blur(blur_w)
    assert R <= MAX_RG, "use kernel() for high-rank blur kernels"
    return _pack_in_maps(ps, AT, BT)


def _run(in_maps, R, trace=False):
    from concourse.bass_utils import run_bass_kernel_spmd
    nc = _get_nc(R)
    return run_bass_kernel_spmd(nc, in_maps, core_ids=list(range(N_CORES)),
                                trace=trace)


def kernel(patch_scores, blur_w, img_h=H, img_w=H, **_ignored):
    assert int(img_h) == H and int(img_w) == H, (img_h, img_w)
    ps = np.asarray(patch_scores, dtype=np.float32).reshape(B_FULL, SH, SH)
    AT, BT, R = _factor_blur(blur_w)
    # high-rank (non-separable) blur kernels don't fit on chip at once:
    # run rank groups of <=MAX_RG and sum the group outputs on the host.
    # The production case (Gaussian blur) is exactly rank 1 -> single pass.
    G = min(R, MAX_RG)
    npass = (R + G - 1) // G
    if npass * G > R:
        pad = np.zeros(((npass * G - R) * SH, H), np.float32)
        AT = np.concatenate([AT, pad], axis=0)
        BT = np.concatenate([BT, pad], axis=0)
    out = None
    for p in range(npass):
        sl = slice(p * G * SH, (p + 1) * G * SH)
        in_maps, _ = _pack_in_maps(ps, AT[sl], BT[sl])
        res = _run(in_maps, G, trace=False)
        # device streams bf16; upcast to f32 on the host
        o = np.concatenate([np.asarray(r["out"]) for r in res.results],
                           axis=0).astype(np.float32)
        out = o if out is None else out + o
    return out.astype(np.float32, copy=False)


# revision 44
# speedup vs baseline: 1.2632x; 1.2632x over previous
"""AnomalyMapGenerator Trainium2 kernel.

Reference computation: nearest-neighbor upsample of patch_scores
[B=32,1,28,28] -> [B,1,512,512], then a dense 33x33 blur conv (padding 16),
then mean over the (singleton) channel dim -> [B,512,512].

Both stages are linear and separable along H and W, so the whole map
collapses to  out[b] = A @ s[b] @ B^T  with A, B of shape [512, 28]:

    up = U s U^T            (U [512,28] is the 0/1 nearest-upsample matrix)
    out = C_h up C_w^T      (C_* [512,512] Toeplitz matrices of the 1-D taps)
    =>  out = (C_h U) s (C_w U)^T = A s B^T

The 33x33 blur weight is factored into separable 1-D taps by SVD on the host
(it is an exact rank-1 Gaussian outer product; general rank-R kernels are
handled by summing rank-1 terms in PSUM). The heavy work - 32 images of
[512,28]@[28,28] and [512,28]@[28,512] matmuls plus the 128 MiB output
write - runs on 8 NeuronCores, batch-sharded 4 images per core.
"""

import numpy as np

# ---- problem geometry (hardcoded per spec) ---------------------------------
B_FULL = 32
SH = 28          # source patch side
H = 512          # output side
KS = 33          # blur kernel side
PAD = KS // 2
N_CORES = 8
PB = B_FULL // N_CORES   # images per core
M_CHUNKS = H // 128      # output row chunks per image
MAX_RG = 4               # max rank-1 blur terms processed per device pass

_cache = {}


def _factor_blur(blur_w):
    """Host-side weight packing: factor the 2-D blur kernel into rank-1
    separable terms and fold each with the nearest-upsample matrix.

    Returns (AT, BT, R): AT/BT are [R*28, 512] f32, where
    AT[r*28:(r+1)*28] = A_r^T and out = sum_r A_r s B_r^T.
    """
    w2d = np.asarray(blur_w, dtype=np.float64).reshape(KS, KS)
    uu, sv, vt = np.linalg.svd(w2d)
    R = max(1, int(np.sum(sv > sv[0] * 1e-6))) if sv[0] > 0 else 1

    idx = np.arange(H)
    U = np.zeros((H, SH))
    U[idx, (idx * SH) // H] = 1.0
    # C[y, Y] = k[Y - y + PAD] for |Y - y| <= PAD (cross-correlation, zero pad)
    D = idx[None, :] - idx[:, None] + PAD
    valid = (D >= 0) & (D <= KS - 1)
    Dc = np.clip(D, 0, KS - 1)

    ats, bts = [], []
    for r in range(R):
        A = np.where(valid, np.take(uu[:, r] * sv[r], Dc), 0.0) @ U   # [512, 28]
        Bm = np.where(valid, np.take(vt[r, :], Dc), 0.0) @ U          # [512, 28]
        ats.append(np.ascontiguousarray(A.T))
        bts.append(np.ascontiguousarray(Bm.T))
    AT = np.concatenate(ats, axis=0).astype(np.float32)  # [R*28, 512]
    BT = np.concatenate(bts, axis=0).astype(np.float32)  # [R*28, 512]
    return AT, BT, R


def _build_nc(R):
    """Per-core Bass graph: out[b] = sum_r A_r s_b B_r^T for PB images.

    mm1:  t_r^T [28,512] = lhsT(s_b [28i,28j]).T @ rhs(A_r^T [28i,512y])
    mm2:  out_c [128,512] += lhsT(t_r^T[:, c*128:+128]).T @ rhs(B_r^T [28j,512x])
    """
    import concourse.mybir as mybir
    from concourse import bacc
    from concourse.tile import TileContext

    f32 = mybir.dt.float32
    bf16 = mybir.dt.bfloat16
    # float32r: same 4-byte storage as f32, but the PE runs it at full rate
    # (1 cycle/row at N>=256) instead of fp32's 4 cycles/row
    f32r = mybir.dt.float32r
    nc = bacc.Bacc("TRN2", target_bir_lowering=False, debug=False,
                   num_devices=N_CORES)

    # packed input [64, R*512 (A^T) | 128 (s, per-pair 32-strided) | R*512
    # (B^T, replicated at partition groups 0 and 32)]: a pair of images runs
    # its two mm2s on disjoint PE row groups concurrently, which requires
    # fmap (B^T) and weights (t^T) to share a base partition. Two DMAs
    # (mm1 operands first) cut the to-first-matmul latency.
    FW = 2 * R * H + 128
    FW1 = R * H + 128
    inp_d = nc.declare_dram_parameter("inp", [64, FW], f32r, isOutput=False)
    # output is staged and streamed to HBM as bf16 (halves the dominant
    # HBM-write cost); the host upcasts to f32. Output quantization error
    # ~2e-3 fro-rel, well inside the accuracy gate.
    out_d = nc.declare_dram_parameter("out", [PB, H, H], bf16, isOutput=True)

    with TileContext(nc) as tc:
        with (
            tc.tile_pool(name="const", bufs=1) as cpool,
            tc.tile_pool(name="tt", bufs=2) as tpool,
            tc.tile_pool(name="pt", bufs=2, space="PSUM") as pt_pool,
            tc.tile_pool(name="po", bufs=6, space="PSUM") as po_pool,
            tc.tile_pool(name="ob", bufs=4) as opool,
        ):
            inp_t = cpool.tile([64, FW], f32r, tag="inp")
            nc.sync.dma_start(out=inp_t[:SH, :FW1], in_=inp_d[:SH, :FW1])
            nc.sync.dma_start(out=inp_t[:, FW1:], in_=inp_d[:, FW1:])
            at_t = inp_t[:SH, 0:R * H]
            s_t = inp_t[:SH, R * H:FW1]  # [28, 128]: pair P at cols P*64
            bt_t = inp_t[:, FW1:]        # [64, R*512]: B^T at groups 0, 32

            for P in range(PB // 2):
                # staging for both images of the pair: (b2, c, x) layout
                ob_t = opool.tile([128, 2 * M_CHUNKS * H], bf16, tag="ob")
                tts = []
                for r in range(R):
                    # one mm1 covers the pair: lhsT [28, 64] -> both t^T at
                    # 32-aligned partition groups of one PSUM tile
                    pt_t = pt_pool.tile([64, H], f32, tag="pt")
                    nc.tensor.matmul(
                        out=pt_t[:],
                        lhsT=s_t[:, P * 64:(P + 1) * 64],
                        rhs=at_t[:, r * H:(r + 1) * H],
                        start=True, stop=True,
                    )
                    tt_t = tpool.tile([64, H], f32r, tag=f"tt{r}")
                    # cast per y-chunk column so each mm2 pair waits on one
                    for c in range(M_CHUNKS):
                        piece = (slice(None), slice(c * 128, (c + 1) * 128))
                        if (c + r) % 2 == 0:
                            nc.vector.tensor_copy(out=tt_t[piece],
                                                  in_=pt_t[piece])
                        else:
                            nc.scalar.copy(out=tt_t[piece], in_=pt_t[piece])
                    tts.append(tt_t)
                for c in range(M_CHUNKS):
                    pos = []
                    for h2 in range(2):  # image P*2 + h2
                        po_t = po_pool.tile([128, H], f32, tag="po",
                                            name=f"po_{P}_{c}_{h2}")
                        for r in range(R):
                            # row groups 0 / 32 -> the pair's two matmuls
                            # execute concurrently in the PE array
                            nc.tensor.matmul(
                                out=po_t[:],
                                lhsT=tts[r][h2 * 32:h2 * 32 + SH,
                                            c * 128:(c + 1) * 128],
                                rhs=bt_t[h2 * 32:h2 * 32 + SH,
                                         r * H:(r + 1) * H],
                                start=(r == 0), stop=(r == R - 1),
                                tile_position=(h2 * 32, 0),
                            )
                        pos.append(po_t)
                    # the pair's two copies run on both engines in parallel,
                    # then the (pair, chunk) leaves as one 256 KiB DMA
                    for h2 in range(2):
                        dst = ob_t[:, (h2 * M_CHUNKS + c) * H:
                                   (h2 * M_CHUNKS + c + 1) * H]
                        if h2 == 0:
                            nc.vector.tensor_copy(out=dst, in_=pos[h2][:])
                        else:
                            nc.scalar.copy(out=dst, in_=pos[h2][:])
                    nc.sync.dma_start(
                        out=out_d[2 * P:2 * P + 2, c * 128:(c + 1) * 128, :]
                            .rearrange("b p x -> p b x"),
                        in_=ob_t[:].rearrange("p (b c x) -> p b c x",
                                              b=2, x=H)[:, :, c, :],
                    )
    nc.compile()
    return nc


def _get_nc(R):
    key = ("nc", R)
    if key not in _cache:
        _cache[key] = _build_nc(R)
    return _cache[key]


def _pack_in_maps(ps, AT, BT):
    """Pack per-core inputs [64, R*512 | 128 | R*512] for one rank group.

    s columns for image b sit at (b//2)*64 + (b%2)*32 + j; B^T is replicated
    at partition groups 0 and 32 for the row-packed mm2 pairs.
    """
    R = AT.shape[0] // SH
    at_cols = np.concatenate([AT[r * SH:(r + 1) * SH] for r in range(R)], axis=1)
    bt_cols = np.concatenate([BT[r * SH:(r + 1) * SH] for r in range(R)], axis=1)
    RH = R * H
    in_maps = []
    for i in range(N_CORES):
        inp = np.zeros((64, 2 * RH + 128), np.float32)
        inp[:SH, :RH] = at_cols
        for b in range(PB):
            col = RH + (b // 2) * 64 + (b % 2) * 32
            inp[:SH, col:col + SH] = ps[i * PB + b]  # [i, j]
        inp[0:SH, RH + 128:] = bt_cols
        inp[32:32 + SH, RH + 128:] = bt_cols
        in_maps.append({"inp": np.ascontiguousarray(inp)})
    return in_maps, R


def _make_in_maps(patch_scores, blur_w):
    ps = np.asarray(patch_scores, dtype=np.float32).reshape(B_FULL, SH, SH)
    AT, BT, R = _factor_blur(blur_w)
    assert R <= MAX_RG, "use kernel() for high-rank blur kernels"
    return _pack_in_maps(ps, AT, BT)


def _run(in_maps, R, trace=False):
    from concourse.bass_utils import run_bass_kernel_spmd
    nc = _get_nc(R)
    return run_bass_kernel_spmd(nc, in_maps, core_ids=list(range(N_CORES)),
                                trace=trace)


def kernel(patch_scores, blur_w, img_h=H, img_w=H, **_ignored):
    assert int(img_h) == H and int(img_w) == H, (img_h, img_w)
    ps = np.asarray(patch_scores, dtype=np.float32).reshape(B_FULL, SH, SH)
    AT, BT, R = _factor_blur(blur_w)
    # high-rank (non-separable) blur kernels don't fit on chip at once:
    # run rank groups of <=MAX_RG and sum the group outputs on the host.
    # The production case (Gaussian blur) is exactly rank 1 -> single pass.
    G = min(R, MAX_RG)
    npass = (R + G - 1) // G
    if npass * G > R:
        pad = np.zeros(((npass * G - R) * SH, H), np.float32)
        AT = np.concatenate([AT, pad], axis=0)
        BT = np.concatenate([BT, pad], axis=0)
    out = None
    for p in range(npass):
        sl = slice(p * G * SH, (p + 1) * G * SH)
        in_maps, _ = _pack_in_maps(ps, AT[sl], BT[sl])
        res = _run(in_maps, G, trace=False)
        # device streams bf16; upcast to f32 on the host
        o = np.concatenate([np.asarray(r["out"]) for r in res.results],
                           axis=0).astype(np.float32)
        out = o if out is None else out + o
    return out.astype(np.float32, copy=False)
